# revision 2
# baseline (speedup 1.0000x reference)
"""DeepSeekMoE (E=8, top-2) on 8 TRN2 NeuronCores — expert-parallel with an
8-way H-split for exact load balance.

Routing runs on host (it IS the data-dependent shard map). Instead of one
expert per core (bottleneck = max expert count, padded), every core processes
ALL eight experts' gathered token sets against its own 512-column slice of
each expert's FFN (w1 columns / w2 rows i*512:(i+1)*512), producing partial
outputs that the host sums during its scatter-add combine. Per-core matmul
work is exactly sum(counts)/8 + 512 shared tokens = 1536 token-FFNs — the
aggregate bf16 roofline — with zero padding. The shared expert stays
data-parallel over 512-token slices (replicated weights) as before.

Device layout: activations transposed ([feature, token]) end to end; both
matmuls use natural-layout weight slices as the stationary operand. bf16
matmuls with f32 PSUM accumulation; gelu+b1 fuses on ScalarE at PSUM
eviction; second-matmul eviction is a plain converting copy on VectorE
(expert b2 bias and top-2 combine weights are applied by the host on the f32
partial sums, so partials need no on-device epilogue). Expert partials return
as bf16 to halve output DMA.

Queueing: weight streams ride the sync queue only (they are never blocked
behind output evictions, so each job's weights land a job early); x
prefetches and output writes ride the Activation HWDGE queue.
"""

import numpy as np
import ml_dtypes

import concourse.mybir as mybir
import concourse.tile as tile
from concourse import bacc
from concourse.bass_utils import run_bass_kernel_spmd

D = 1024
E = 8
TOPK = 2
H = 4096
NCORES = 8
P = 128
Q = H // NCORES      # 512 H-columns handled per core per expert
NCHUNK = 512         # PSUM-bank token chunk
ND = D // P          # 8
NJ = Q // P          # 4
NH = H // P          # 32
NHQ = H // 512       # 8

BF16 = mybir.dt.bfloat16
F32 = mybir.dt.float32

_cache: dict = {}


def _chunks(c):
    """Near-equal chunks of width <= NCHUNK covering c tokens."""
    nch = -(-c // NCHUNK)
    base, rem = divmod(c, nch)
    out, o = [], 0
    for i in range(nch):
        n = base + (1 if i < rem else 0)
        out.append((o, n))
        o += n
    return out


def build(counts, S: int):
    """Build + compile the SPMD per-core program.

    counts: exact per-expert token counts (same on all cores; data-dependent).
    S: shared-expert tokens per core.
    """
    nc = bacc.Bacc(None, target_bir_lowering=False, debug=False)

    xg = [nc.declare_dram_parameter(f"xg{e}", [D, counts[e]], BF16,
                                    isOutput=False) for e in range(E)]
    w1q = [nc.declare_dram_parameter(f"w1q{e}", [ND, P, Q], BF16,
                                     isOutput=False) for e in range(E)]
    w2q = [nc.declare_dram_parameter(f"w2q{e}", [NJ, P, D], BF16,
                                     isOutput=False) for e in range(E)]
    b1q = nc.declare_dram_parameter("b1q", [P, 4 * E], F32, isOutput=False)
    sx = nc.declare_dram_parameter("sx", [D, S], BF16, isOutput=False)
    sw1 = nc.declare_dram_parameter("sw1", [NHQ, ND, P, 512], BF16, isOutput=False)
    sw2 = nc.declare_dram_parameter("sw2", [ND, NHQ, P, 4, P], BF16, isOutput=False)
    sb1c = nc.declare_dram_parameter("sb1c", [P, NH], F32, isOutput=False)
    sb2c = nc.declare_dram_parameter("sb2c", [P, ND], F32, isOutput=False)
    ye = [nc.declare_dram_parameter(f"ye{e}", [D, counts[e]], BF16,
                                    isOutput=True) for e in range(E)]
    ys = nc.declare_dram_parameter("ys", [D, S], F32, isOutput=True)

    with tile.TileContext(nc) as tc:
        with (
            tc.tile_pool(name="wp1", bufs=16) as wp1,
            tc.tile_pool(name="wp2e", bufs=8) as wp2e,
            tc.tile_pool(name="wp2s", bufs=16) as wp2s,
            tc.tile_pool(name="xp", bufs=16) as xp,
            tc.tile_pool(name="sp", bufs=8) as sp,
            tc.tile_pool(name="hp", bufs=8) as hp,
            tc.tile_pool(name="hps", bufs=32) as hps,
            tc.tile_pool(name="cp", bufs=1) as cp,
            tc.tile_pool(name="op", bufs=4) as op,
            tc.tile_pool(name="pp", bufs=8, space="PSUM") as pp,
        ):
            b1t = cp.tile([P, 4 * E], F32, tag="b1")
            sb1t = cp.tile([P, NH], F32, tag="sb1")
            sb2t = cp.tile([P, ND], F32, tag="sb2")
            sxt = [sp.tile([P, S], BF16, tag="sx", name=f"sx{d}")
                   for d in range(ND)]
            xts = {}

            def load_x(e):
                ts = []
                for d in range(ND):
                    t = xp.tile([P, counts[e]], BF16, tag="x", name=f"x{e}_{d}")
                    nc.scalar.dma_start(t[:], xg[e][d * P:(d + 1) * P, :])
                    ts.append(t)
                xts[e] = ts

            def expert_job(e):
                C = counts[e]
                chs = _chunks(C)
                # weight streams on sync queue: this job's w1 (needed first),
                # this job's w2 (needed at p2), then nothing until next job —
                # so job e+1's weights transfer during job e's compute.
                w1ts = []
                for d in range(ND):
                    t = wp1.tile([P, Q], BF16, tag="w1", name=f"w1_{e}_{d}")
                    nc.sync.dma_start(t[:], w1q[e][d])
                    w1ts.append(t)
                    if e == 0:
                        if d == 0:
                            nc.scalar.dma_start(b1t[:], b1q[:])
                        t2 = xp.tile([P, C], BF16, tag="x", name=f"x0_{d}")
                        nc.scalar.dma_start(t2[:], xg[0][d * P:(d + 1) * P, :])
                        xts.setdefault(0, []).append(t2)
                w2ts = []
                for j in range(NJ):
                    t = wp2e.tile([P, D], BF16, tag="w2e", name=f"w2_{e}_{j}")
                    nc.sync.dma_start(t[:], w2q[e][j])
                    w2ts.append(t)
                # prefetch next job's activations on the scalar queue
                if e + 1 < E:
                    load_x(e + 1)
                else:
                    for d in range(ND):
                        nc.scalar.dma_start(sxt[d][:], sx[d * P:(d + 1) * P, :])
                    nc.scalar.dma_start(sb1t[:], sb1c[:])
                    nc.scalar.dma_start(sb2t[:], sb2c[:])

                # phase 1: h[quad rows, tok] = gelu(w1q.T @ x + b1q)
                hts = []
                for hh in range(NJ):
                    ht = hp.tile([P, C], BF16, tag="he", name=f"h{e}_{hh}")
                    psums = [pp.tile([P, n], F32, tag="ps", name=f"psA{e}_{hh}_{ti}")
                             for ti, (_, n) in enumerate(chs)]
                    for d in range(ND):
                        for ti, (o, n) in enumerate(chs):
                            nc.tensor.matmul(
                                psums[ti][:, :n],
                                w1ts[d][:, hh * P:(hh + 1) * P],
                                xts[e][d][:, o:o + n],
                                start=(d == 0),
                                stop=(d == ND - 1),
                            )
                    for ti, (o, n) in enumerate(chs):
                        nc.scalar.activation(
                            ht[:, o:o + n],
                            psums[ti][:, :n],
                            mybir.ActivationFunctionType.Gelu,
                            bias=b1t[:, e * 4 + hh:e * 4 + hh + 1],
                        )
                    hts.append(ht)
                del xts[e]

                # phase 2: ye_partial[dt, tok] = w2q[:, dt].T @ h  (bf16 out;
                # b2 and combine weights applied on host)
                for dt in range(ND):
                    psums = [pp.tile([P, n], F32, tag="ps", name=f"psB{e}_{dt}_{ti}")
                             for ti, (_, n) in enumerate(chs)]
                    for j in range(NJ):
                        for ti, (o, n) in enumerate(chs):
                            nc.tensor.matmul(
                                psums[ti][:, :n],
                                w2ts[j][:, dt * P:(dt + 1) * P],
                                hts[j][:, o:o + n],
                                start=(j == 0),
                                stop=(j == NJ - 1),
                            )
                    ot = op.tile([P, C], BF16, tag="o", name=f"o{e}_{dt}")
                    for ti, (o, n) in enumerate(chs):
                        nc.vector.tensor_scalar_add(
                            ot[:, o:o + n], psums[ti][:, :n], 0.0)
                    nc.scalar.dma_start(ye[e][dt * P:(dt + 1) * P, :], ot[:, :C])

            def shared_job():
                # full shared-expert FFN over this core's S-token slice,
                # weights streamed JIT on the sync queue (baseline pattern)
                hts = []
                for hq in range(NHQ):
                    w1ts = []
                    for d in range(ND):
                        t = wp1.tile([P, 512], BF16, tag="w1", name=f"sw1_{hq}_{d}")
                        nc.sync.dma_start(t[:], sw1[hq, d])
                        w1ts.append(t)
                    for hh in range(4):
                        h = hq * 4 + hh
                        ht = hps.tile([P, S], BF16, tag="hs", name=f"sh{h}")
                        ps = pp.tile([P, S], F32, tag="ps", name=f"psS1_{h}")
                        for d in range(ND):
                            nc.tensor.matmul(
                                ps[:, :S],
                                w1ts[d][:, hh * P:(hh + 1) * P],
                                sxt[d][:, :S],
                                start=(d == 0),
                                stop=(d == ND - 1),
                            )
                        nc.scalar.activation(
                            ht[:, :S], ps[:, :S],
                            mybir.ActivationFunctionType.Gelu,
                            bias=sb1t[:, h:h + 1],
                        )
                        hts.append(ht)
                for dt in range(ND):
                    w2ts = []
                    for j in range(NHQ):
                        t = wp2s.tile([P, 4, P], BF16, tag="w2s", name=f"sw2_{dt}_{j}")
                        nc.sync.dma_start(t[:], sw2[dt, j])
                        w2ts.append(t)
                    ps = pp.tile([P, S], F32, tag="ps", name=f"psS2_{dt}")
                    for h in range(NH):
                        j, a = divmod(h, 4)
                        nc.tensor.matmul(
                            ps[:, :S],
                            w2ts[j][:, a, :],
                            hts[h][:, :S],
                            start=(h == 0),
                            stop=(h == NH - 1),
                        )
                    ot = op.tile([P, S], F32, tag="os", name=f"os{dt}")
                    nc.vector.tensor_scalar_add(ot[:, :S], ps[:, :S],
                                                sb2t[:, dt:dt + 1])
                    nc.scalar.dma_start(ys[dt * P:(dt + 1) * P, :], ot[:, :S])

            for e in range(E):
                expert_job(e)
            shared_job()

    nc.compile()
    return nc


def _get_nc(counts, S):
    key = (tuple(counts), S)
    if key not in _cache:
        _cache[key] = build(tuple(counts), S)
    return _cache[key]


def _pack_w1(w):
    # [D, H] -> [hq, d, 128, 512]
    return np.ascontiguousarray(
        np.asarray(w).reshape(D // P, P, H // 512, 512).transpose(2, 0, 1, 3)
    ).astype(ml_dtypes.bfloat16)


def _pack_w2(w):
    # [H, D] -> [dt, j, 128, 4, 128]
    return np.ascontiguousarray(
        np.asarray(w).reshape(H // 512, 4, P, D // P, P).transpose(3, 0, 2, 1, 4)
    ).astype(ml_dtypes.bfloat16)


def prepare(x, gate_w, gate_b, route_bias, shared_w1, shared_b1, shared_w2,
            shared_b2, exp_w1, exp_b1, exp_w2, exp_b2):
    """Host routing + sharding. Returns (nc, in_maps, combine_fn)."""
    B, SEQ, _ = x.shape
    T = B * SEQ
    S = T // NCORES
    xf = np.ascontiguousarray(x.reshape(T, D)).astype(np.float32)

    # --- gate / routing (this IS the data-dependent shard map) ---
    logits = xf @ np.asarray(gate_w, np.float32) + np.asarray(gate_b, np.float32) \
        + np.asarray(route_bias, np.float32)
    m = logits.max(axis=-1, keepdims=True)
    e = np.exp(logits - m)
    probs = e / e.sum(axis=-1, keepdims=True)
    i1 = probs.argmax(axis=-1)
    p1 = probs[np.arange(T), i1]
    probs2 = probs.copy()
    probs2[np.arange(T), i1] = -np.inf
    i2 = probs2.argmax(axis=-1)
    p2 = probs[np.arange(T), i2]
    den = p1 + p2
    p1n = p1 / den
    p2n = p2 / den

    idx = []
    pv = []
    for ex in range(E):
        sel1 = np.nonzero(i1 == ex)[0]
        sel2 = np.nonzero(i2 == ex)[0]
        idx.append(np.concatenate([sel1, sel2]))
        pv.append(np.concatenate([p1n[sel1], p2n[sel2]]).astype(np.float32))
    counts = [len(ix) for ix in idx]

    xf_bf = xf.astype(ml_dtypes.bfloat16)
    xg_e = [np.ascontiguousarray(xf_bf[idx[ex]].T) for ex in range(E)]
    sw1_p = _pack_w1(shared_w1)
    sw2_p = _pack_w2(shared_w2)
    sb1c = np.ascontiguousarray(np.asarray(shared_b1, np.float32).reshape(H // P, P).T)
    sb2c = np.ascontiguousarray(np.asarray(shared_b2, np.float32).reshape(D // P, P).T)
    ew1 = [np.asarray(exp_w1[ex], np.float32) for ex in range(E)]
    ew2 = [np.asarray(exp_w2[ex], np.float32) for ex in range(E)]
    eb1 = [np.asarray(exp_b1[ex], np.float32) for ex in range(E)]
    eb2 = [np.asarray(exp_b2[ex], np.float32) for ex in range(E)]

    in_maps = []
    for c in range(NCORES):
        lo, hi = c * Q, (c + 1) * Q
        im = {
            "b1q": np.ascontiguousarray(np.concatenate(
                [eb1[ex][lo:hi].reshape(NJ, P).T for ex in range(E)], axis=1)),
            "sx": np.ascontiguousarray(xf_bf[c * S:(c + 1) * S].T),
            "sw1": sw1_p,
            "sw2": sw2_p,
            "sb1c": sb1c,
            "sb2c": sb2c,
        }
        for ex in range(E):
            im[f"xg{ex}"] = xg_e[ex]
            im[f"w1q{ex}"] = np.ascontiguousarray(
                ew1[ex][:, lo:hi].reshape(ND, P, Q).astype(ml_dtypes.bfloat16))
            im[f"w2q{ex}"] = np.ascontiguousarray(
                ew2[ex][lo:hi, :].reshape(NJ, P, D).astype(ml_dtypes.bfloat16))
        in_maps.append(im)

    nc = _get_nc(counts, S)

    def combine(results):
        out = np.zeros((T, D), np.float32)
        for c in range(NCORES):
            out[c * S:(c + 1) * S] = results[c]["ys"].T
        for ex in range(E):
            ysum = results[0][f"ye{ex}"].astype(np.float32)
            for c in range(1, NCORES):
                ysum += results[c][f"ye{ex}"].astype(np.float32)
            out[idx[ex]] += (ysum.T + eb2[ex][None, :]) * pv[ex][:, None]
        return out.reshape(B, SEQ, D)

    return nc, in_maps, combine


def kernel(**inputs):
    nc, in_maps, combine = prepare(**inputs)
    res = run_bass_kernel_spmd(nc, in_maps, core_ids=list(range(NCORES)))
    return combine(res.results)


# revision 3
# speedup vs baseline: 1.0324x; 1.0324x over previous
"""DeepSeekMoE (E=8, top-2) on 8 TRN2 NeuronCores — expert-parallel with an
8-way H-split for exact load balance.

Routing runs on host (it IS the data-dependent shard map). Instead of one
expert per core (bottleneck = max expert count, padded), every core processes
ALL eight experts' gathered token sets against its own 512-column slice of
each expert's FFN (w1 columns / w2 rows i*512:(i+1)*512), producing partial
outputs the host sums during its scatter-add combine. Per-core matmul work is
exactly sum(counts)/8 + 512 shared tokens = 1536 token-FFNs — the aggregate
bf16 roofline — with zero padding. The shared expert stays data-parallel over
512-token slices (replicated weights).

Device layout: activations transposed ([feature, token]); bf16 matmuls with
f32 PSUM accumulation; gelu+b1 fused on ScalarE at PSUM eviction; second
matmul evicts via a converting copy on VectorE (expert b2 and the top-2
combine weights are applied by the host on the f32 partial sums). Partials
return as bf16 to halve output DMA.

DMA: every operand is packed on host so one transfer = one big 2D descriptor
with >=8KB contiguous per partition line (small per-partition lines measure
~3x slower per byte). Three queues: weights on sync HWDGE, activations +
biases on the Activation HWDGE, outputs on the gpsimd software DGE — so
weight streams are never blocked behind output evictions and x prefetches
never queue behind weight bulk.
"""

import numpy as np
import ml_dtypes

import concourse.mybir as mybir
import concourse.tile as tile
from concourse import bacc
from concourse.bass_utils import run_bass_kernel_spmd

D = 1024
E = 8
TOPK = 2
H = 4096
NCORES = 8
P = 128
Q = H // NCORES      # 512 H-columns handled per core per expert
NCHUNK = 512         # PSUM-bank token chunk
ND = D // P          # 8
NJ = Q // P          # 4
NH = H // P          # 32
NHQ = H // 512       # 8

BF16 = mybir.dt.bfloat16
F32 = mybir.dt.float32

_cache: dict = {}


def _chunks(c):
    """Near-equal chunks of width <= NCHUNK covering c tokens."""
    nch = -(-c // NCHUNK)
    base, rem = divmod(c, nch)
    out, o = [], 0
    for i in range(nch):
        n = base + (1 if i < rem else 0)
        out.append((o, n))
        o += n
    return out


def build(counts, S: int):
    """Build + compile the SPMD per-core program.

    counts: exact per-expert token counts (same on all cores; data-dependent).
    S: shared-expert tokens per core.
    """
    nc = bacc.Bacc(None, target_bir_lowering=False, debug=False)

    xg = [nc.declare_dram_parameter(f"xg{e}", [P, ND * counts[e]], BF16,
                                    isOutput=False) for e in range(E)]
    w1q = [nc.declare_dram_parameter(f"w1q{e}", [P, ND * Q], BF16,
                                     isOutput=False) for e in range(E)]
    w2q = [nc.declare_dram_parameter(f"w2q{e}", [P, NJ * D], BF16,
                                     isOutput=False) for e in range(E)]
    b1q = nc.declare_dram_parameter("b1q", [P, NJ * E], F32, isOutput=False)
    sx = nc.declare_dram_parameter("sx", [P, ND * S], BF16, isOutput=False)
    sw1 = nc.declare_dram_parameter("sw1", [NHQ, P, ND * 512], BF16, isOutput=False)
    sw2 = nc.declare_dram_parameter("sw2", [ND, P, NH * P], BF16, isOutput=False)
    sb1c = nc.declare_dram_parameter("sb1c", [P, NH], F32, isOutput=False)
    sb2c = nc.declare_dram_parameter("sb2c", [P, ND], F32, isOutput=False)
    ye = [nc.declare_dram_parameter(f"ye{e}", [P, ND * counts[e]], BF16,
                                    isOutput=True) for e in range(E)]
    ys = nc.declare_dram_parameter("ys", [P, ND * S], BF16, isOutput=True)

    with tile.TileContext(nc) as tc:
        with (
            tc.tile_pool(name="wp1", bufs=3) as wp1,
            tc.tile_pool(name="wp2", bufs=3) as wp2,
            tc.tile_pool(name="xp", bufs=2) as xp,
            tc.tile_pool(name="hp", bufs=8) as hp,
            tc.tile_pool(name="hps", bufs=32) as hps,
            tc.tile_pool(name="cp", bufs=1) as cp,
            tc.tile_pool(name="op", bufs=2) as op,
            tc.tile_pool(name="pp", bufs=8, space="PSUM") as pp,
        ):
            b1t = cp.tile([P, NJ * E], F32, tag="b1")
            sb1t = cp.tile([P, NH], F32, tag="sb1")
            sb2t = cp.tile([P, ND], F32, tag="sb2")
            sxt = cp.tile([P, ND * S], BF16, tag="sx")
            xts = {}

            def load_x(e):
                C = counts[e]
                t = xp.tile([P, ND * C], BF16, tag="x", name=f"x{e}")
                if e == 0:
                    # split so the d=0 slice lands fast at t=0
                    nc.scalar.dma_start(b1t[:], b1q[:])
                    for d in range(ND):
                        nc.scalar.dma_start(t[:, d * C:(d + 1) * C],
                                            xg[e][:, d * C:(d + 1) * C])
                else:
                    nc.scalar.dma_start(t[:], xg[e][:])
                xts[e] = t

            def expert_job(e):
                C = counts[e]
                chs = _chunks(C)
                # weights on the sync queue: two descriptors per job, so the
                # stream runs a full job ahead of compute
                w1t = wp1.tile([P, ND * Q], BF16, tag="w1", name=f"w1_{e}")
                if e == 0:
                    load_x(0)
                    for d in range(ND):
                        nc.sync.dma_start(w1t[:, d * Q:(d + 1) * Q],
                                          w1q[e][:, d * Q:(d + 1) * Q])
                else:
                    nc.sync.dma_start(w1t[:], w1q[e][:])
                w2t = wp2.tile([P, NJ * D], BF16, tag="w2", name=f"w2_{e}")
                nc.sync.dma_start(w2t[:], w2q[e][:])
                # prefetch next job's activations on the scalar queue
                if e + 1 < E:
                    load_x(e + 1)
                else:
                    nc.scalar.dma_start(sxt[:], sx[:])
                    nc.scalar.dma_start(sb1t[:], sb1c[:])
                    nc.scalar.dma_start(sb2t[:], sb2c[:])

                xt = xts[e]
                # phase 1: h[quad rows, tok] = gelu(w1q.T @ x + b1q)
                hts = []
                for hh in range(NJ):
                    ht = hp.tile([P, C], BF16, tag="he", name=f"h{e}_{hh}")
                    psums = [pp.tile([P, n], F32, tag="ps", name=f"psA{e}_{hh}_{ti}")
                             for ti, (_, n) in enumerate(chs)]
                    for d in range(ND):
                        for ti, (o, n) in enumerate(chs):
                            nc.tensor.matmul(
                                psums[ti][:, :n],
                                w1t[:, d * Q + hh * P:d * Q + (hh + 1) * P],
                                xt[:, d * C + o:d * C + o + n],
                                start=(d == 0),
                                stop=(d == ND - 1),
                            )
                    for ti, (o, n) in enumerate(chs):
                        nc.scalar.activation(
                            ht[:, o:o + n],
                            psums[ti][:, :n],
                            mybir.ActivationFunctionType.Gelu,
                            bias=b1t[:, e * NJ + hh:e * NJ + hh + 1],
                        )
                    hts.append(ht)
                del xts[e]

                # phase 2: ye_partial[dt, tok] = w2q[:, dt].T @ h  (bf16 out;
                # b2 and combine weights applied on host)
                ot = op.tile([P, ND * C], BF16, tag="o", name=f"o{e}")
                for dt in range(ND):
                    psums = [pp.tile([P, n], F32, tag="ps", name=f"psB{e}_{dt}_{ti}")
                             for ti, (_, n) in enumerate(chs)]
                    for j in range(NJ):
                        for ti, (o, n) in enumerate(chs):
                            nc.tensor.matmul(
                                psums[ti][:, :n],
                                w2t[:, j * D + dt * P:j * D + (dt + 1) * P],
                                hts[j][:, o:o + n],
                                start=(j == 0),
                                stop=(j == NJ - 1),
                            )
                    for ti, (o, n) in enumerate(chs):
                        nc.vector.tensor_scalar_add(
                            ot[:, dt * C + o:dt * C + o + n],
                            psums[ti][:, :n], 0.0)
                nc.gpsimd.dma_start(ye[e][:], ot[:])

            def shared_job():
                # full shared-expert FFN over this core's S-token slice,
                # weights streamed JIT on the sync queue
                hts = []
                for hq in range(NHQ):
                    w1t = wp1.tile([P, ND * 512], BF16, tag="w1", name=f"sw1_{hq}")
                    nc.sync.dma_start(w1t[:], sw1[hq])
                    for hh in range(4):
                        h = hq * 4 + hh
                        ht = hps.tile([P, S], BF16, tag="hs", name=f"sh{h}")
                        ps = pp.tile([P, S], F32, tag="ps", name=f"psS1_{h}")
                        for d in range(ND):
                            nc.tensor.matmul(
                                ps[:, :S],
                                w1t[:, d * 512 + hh * P:d * 512 + (hh + 1) * P],
                                sxt[:, d * S:d * S + S],
                                start=(d == 0),
                                stop=(d == ND - 1),
                            )
                        nc.scalar.activation(
                            ht[:, :S], ps[:, :S],
                            mybir.ActivationFunctionType.Gelu,
                            bias=sb1t[:, h:h + 1],
                        )
                        hts.append(ht)
                ot = op.tile([P, ND * S], BF16, tag="o", name="os")
                for dt in range(ND):
                    w2t = wp2.tile([P, NH * P], BF16, tag="w2", name=f"sw2_{dt}")
                    nc.sync.dma_start(w2t[:], sw2[dt])
                    ps = pp.tile([P, S], F32, tag="ps", name=f"psS2_{dt}")
                    for h in range(NH):
                        nc.tensor.matmul(
                            ps[:, :S],
                            w2t[:, h * P:(h + 1) * P],
                            hts[h][:, :S],
                            start=(h == 0),
                            stop=(h == NH - 1),
                        )
                    nc.vector.tensor_scalar_add(
                        ot[:, dt * S:dt * S + S], ps[:, :S],
                        sb2t[:, dt:dt + 1])
                nc.gpsimd.dma_start(ys[:], ot[:])

            for e in range(E):
                expert_job(e)
            shared_job()

    nc.compile()
    return nc


def _get_nc(counts, S):
    key = (tuple(counts), S)
    if key not in _cache:
        _cache[key] = build(tuple(counts), S)
    return _cache[key]


def _pack_fm(a):
    """[D', N] feature-major -> [P, (D'/P)*N] single-descriptor layout."""
    dp, n = a.shape
    return np.ascontiguousarray(
        a.reshape(dp // P, P, n).transpose(1, 0, 2).reshape(P, (dp // P) * n))


def prepare(x, gate_w, gate_b, route_bias, shared_w1, shared_b1, shared_w2,
            shared_b2, exp_w1, exp_b1, exp_w2, exp_b2):
    """Host routing + sharding. Returns (nc, in_maps, combine_fn)."""
    B, SEQ, _ = x.shape
    T = B * SEQ
    S = T // NCORES
    xf = np.ascontiguousarray(x.reshape(T, D)).astype(np.float32)

    # --- gate / routing (this IS the data-dependent shard map) ---
    logits = xf @ np.asarray(gate_w, np.float32) + np.asarray(gate_b, np.float32) \
        + np.asarray(route_bias, np.float32)
    m = logits.max(axis=-1, keepdims=True)
    e = np.exp(logits - m)
    probs = e / e.sum(axis=-1, keepdims=True)
    i1 = probs.argmax(axis=-1)
    p1 = probs[np.arange(T), i1]
    probs2 = probs.copy()
    probs2[np.arange(T), i1] = -np.inf
    i2 = probs2.argmax(axis=-1)
    p2 = probs[np.arange(T), i2]
    den = p1 + p2
    p1n = p1 / den
    p2n = p2 / den

    idx = []
    pv = []
    for ex in range(E):
        sel1 = np.nonzero(i1 == ex)[0]
        sel2 = np.nonzero(i2 == ex)[0]
        idx.append(np.concatenate([sel1, sel2]))
        pv.append(np.concatenate([p1n[sel1], p2n[sel2]]).astype(np.float32))
    counts = [len(ix) for ix in idx]

    xf_bf = xf.astype(ml_dtypes.bfloat16)
    xg_e = [_pack_fm(np.ascontiguousarray(xf_bf[idx[ex]].T)) for ex in range(E)]
    sw1_p = np.stack([_pack_fm(np.asarray(shared_w1, np.float32)
                               [:, hq * 512:(hq + 1) * 512]
                               .astype(ml_dtypes.bfloat16))
                      for hq in range(NHQ)])
    # sw2[dt] = [P, NH*P] with (p, h*P+c) = shared_w2[h*128+p, dt*128+c]
    sw2_p = np.ascontiguousarray(
        np.asarray(shared_w2, np.float32).astype(ml_dtypes.bfloat16)
        .reshape(NH, P, ND, P).transpose(2, 1, 0, 3).reshape(ND, P, NH * P))
    sb1c = np.ascontiguousarray(np.asarray(shared_b1, np.float32).reshape(H // P, P).T)
    sb2c = np.ascontiguousarray(np.asarray(shared_b2, np.float32).reshape(D // P, P).T)
    ew1 = [np.asarray(exp_w1[ex], np.float32) for ex in range(E)]
    ew2 = [np.asarray(exp_w2[ex], np.float32) for ex in range(E)]
    eb1 = [np.asarray(exp_b1[ex], np.float32) for ex in range(E)]
    eb2 = [np.asarray(exp_b2[ex], np.float32) for ex in range(E)]

    in_maps = []
    for c in range(NCORES):
        lo, hi = c * Q, (c + 1) * Q
        im = {
            "b1q": np.ascontiguousarray(np.concatenate(
                [eb1[ex][lo:hi].reshape(NJ, P).T for ex in range(E)], axis=1)),
            "sx": _pack_fm(np.ascontiguousarray(xf_bf[c * S:(c + 1) * S].T)),
            "sw1": sw1_p,
            "sw2": sw2_p,
            "sb1c": sb1c,
            "sb2c": sb2c,
        }
        for ex in range(E):
            im[f"xg{ex}"] = xg_e[ex]
            im[f"w1q{ex}"] = _pack_fm(ew1[ex][:, lo:hi].astype(ml_dtypes.bfloat16))
            im[f"w2q{ex}"] = np.ascontiguousarray(
                ew2[ex][lo:hi, :].astype(ml_dtypes.bfloat16)
                .reshape(NJ, P, D).transpose(1, 0, 2).reshape(P, NJ * D))
        in_maps.append(im)

    nc = _get_nc(counts, S)

    def unpack_fm(a, n):
        # [P, ND*n] -> [D, n]
        return a.reshape(P, ND, n).transpose(1, 0, 2).reshape(D, n)

    def combine(results):
        out = np.zeros((T, D), np.float32)
        for c in range(NCORES):
            out[c * S:(c + 1) * S] = unpack_fm(
                results[c]["ys"].astype(np.float32), S).T
        for ex in range(E):
            n = counts[ex]
            ysum = results[0][f"ye{ex}"].astype(np.float32)
            for c in range(1, NCORES):
                ysum += results[c][f"ye{ex}"].astype(np.float32)
            out[idx[ex]] += (unpack_fm(ysum, n).T + eb2[ex][None, :]) \
                * pv[ex][:, None]
        return out.reshape(B, SEQ, D)

    return nc, in_maps, combine


def kernel(**inputs):
    nc, in_maps, combine = prepare(**inputs)
    res = run_bass_kernel_spmd(nc, in_maps, core_ids=list(range(NCORES)))
    return combine(res.results)
